# revision 8
# baseline (speedup 1.0000x reference)
"""GroupLinear Trainium2 kernel.

out[b, g, o] = sum_i x[b, i] * W[g, o, i] + b[g, o]
  x: (4096, 1024) f32, W: (16, 1024, 1024) f32, b: (16, 1024) f32
  out: (4096, 16, 1024) f32

Sharding: groups across the 8 cores (2 groups/core), x replicated.

Per-core schedule (all operands pre-transposed host-side, cast to bf16):
  - x is staged as xt[m][i_part, kt*128 + b] (contraction dim on partitions),
    W as wt[i_part, kt, g*1024+o]. No on-device transposes at all.
  - For each batch tile m: 8 kt-steps x 4 psum chunks of 512 accumulate
    out[b, go] with x-tile stationary / W moving; bias fused into the DVE
    PSUM->SBUF evacuation.
  - A post-finalize pass deletes InstLdweights that reload the stationary
    operand already resident in the PE array (walrus emits one per matmul;
    consecutive matmuls here share lhsT so 3 of 4 reloads are redundant --
    each costs ~50-150ns of serialized PE time).
"""

import sys
import types

sys.path.insert(0, "/opt/trn_rl_repo")

# Provide antenv.axon_hooks (NTFF profile hook registry) if the installed
# antenv lacks it — must exist before the first jax/axon backend init.
try:
    from antenv import axon_hooks as _axon_hooks  # noqa: F401
except ImportError:
    _m = types.ModuleType("antenv.axon_hooks")
    _m._hook = None

    def _set_hook(hook, _m=_m):
        _m._hook = hook

    def _get_hook(_m=_m):
        return _m._hook

    _m.set_axon_ntff_profile_hook = _set_hook
    _m.get_axon_ntff_profile_hook = _get_hook
    sys.modules["antenv.axon_hooks"] = _m
    try:
        import antenv

        antenv.axon_hooks = _m
    except ImportError:
        pass

from contextlib import ExitStack

import ml_dtypes
import numpy as np

import concourse.mybir as mybir
import concourse.tile as tile
from concourse import bacc
from concourse.bass_utils import run_bass_kernel_spmd

F32 = mybir.dt.float32
BF16 = mybir.dt.bfloat16

BATCH, D_IN, D_OUT, GROUPS, NCORES = 4096, 1024, 1024, 16, 8
GPC = GROUPS // NCORES  # groups per core
P = 128
KT = D_IN // P          # contraction tiles
MT = BATCH // P         # batch tiles
GO = GPC * D_OUT        # output columns per core
CW = 512                # psum chunk width (1 bank fp32)
NCH = GO // CW          # psum chunks per batch tile


def dedup_ldweights(nc):
    """Delete InstLdweights that reload the exact stationary operand already
    resident in the PE array (same AP as the previous PE Ldweights, nothing
    in between that could clobber the array), when they carry no syncs."""
    n_removed = 0
    for blk in nc.m.functions[0].blocks:
        last_key = None
        to_remove = []
        for inst in blk.instructions:
            if getattr(inst, "engine", None) != mybir.EngineType.PE:
                continue
            tn = type(inst).__name__
            if tn == "InstLdweights":
                ap = inst.ins[0]
                key = (ap.memref, ap.offset, str(ap.ap), str(ap.dtype))
                si = inst.sync_info
                has_sync = si is not None and (
                    len(si.on_wait) > 0 or len(si.on_update) > 0
                )
                if key == last_key and not has_sync:
                    to_remove.append(inst)
                    n_removed += 1
                last_key = key
            elif tn == "InstMatmult" and inst.ldweights is False:
                continue  # non-self-loading matmul: array weights unchanged
            elif tn == "InstEventSemaphore":
                continue  # pure sync, does not touch the array
            else:
                last_key = None  # anything else: conservatively assume clobber
        for inst in to_remove:
            blk.instructions.remove(inst)
    return n_removed


def build_nc():
    nc = bacc.Bacc("TRN2", target_bir_lowering=False, debug=False)
    x = nc.dram_tensor("x", [MT // 2, P, 2 * KT * P], BF16, kind="ExternalInput").ap()
    W = nc.dram_tensor("W", [P, KT, GO], BF16, kind="ExternalInput").ap()
    b = nc.dram_tensor("b", [1, GO], F32, kind="ExternalInput").ap()
    out = nc.dram_tensor("out", [BATCH, GO], F32, kind="ExternalOutput").ap()

    with ExitStack() as ctx:
        tc = ctx.enter_context(tile.TileContext(nc))
        singles = ctx.enter_context(tc.tile_pool(name="singles", bufs=1))
        wt_pool = ctx.enter_context(tc.tile_pool(name="wt", bufs=1))
        xin_pool = ctx.enter_context(tc.tile_pool(name="xin", bufs=3))  # m-pair tiles
        out_pool = ctx.enter_context(tc.tile_pool(name="outp", bufs=3))
        ps_mm = ctx.enter_context(tc.tile_pool(name="ps_mm", bufs=8, space="PSUM"))

        import concourse.bass as bass

        # DMA on TRN2 is descriptor-paced (~46ns per partition-row per
        # queue), so transfers are organized for fat contiguous rows:
        # W as two 4-kt quads (16KB rows), x as m-pair tiles (4KB rows),
        # out stores split by partition halves across two queues.

        # W resident in SBUF: one tile per kt, loads alternating between the
        # sync and gpsimd queues so chunks land in consumption order at the
        # aggregate HBM rate while matmuls chase them
        wts = []
        for kt in range(KT):
            w_sb = wt_pool.tile([P, GO], BF16, tag=f"wt{kt}")
            eng = nc.sync if kt % 2 == 0 else nc.gpsimd
            eng.dma_start(out=w_sb[:, :], in_=W[:, kt, :])
            wts.append(w_sb)

        def wslice(kt, c):
            return wts[kt][:, c * CW : (c + 1) * CW]

        NPAIR = MT // 2

        def load_xpair(pr):
            x_sb = xin_pool.tile([P, 2 * KT * P], BF16, tag="xin")
            nc.scalar.dma_start(out=x_sb[:, :], in_=x[pr, :, :])
            return x_sb

        x_pairs = {0: load_xpair(0)}

        # bias broadcast to all 128 partitions; on the scalar queue after the
        # first x pair (needed only when the first evacuation runs ~20us in,
        # and keeping it off sync/gpsimd lets the W chunks stream at rate)
        bias_sb = singles.tile([P, GO], F32)
        b_bcast = bass.AP(tensor=b.tensor, offset=b.offset, ap=[[0, P], [1, GO]])
        nc.scalar.dma_start(out=bias_sb[:, :], in_=b_bcast)

        # HAM warmup: dummy matmuls on a zeroed tile while W/x stream in, so
        # the PE clock-gate is at 8/8 by the time real matmuls start.
        warm = singles.tile([P, CW], BF16)
        nc.vector.memset(warm[:, :], 0.0)
        warm_ps = ps_mm.tile([P, CW], F32, tag="ps", name="ps_warm")
        for _ in range(12):
            nc.tensor.matmul(
                warm_ps[:, :], warm[:, 0:P], warm[:, :], start=True, stop=True
            )

        for m in range(MT):
            pr, half = divmod(m, 2)
            if half == 0 and pr + 1 < NPAIR:
                x_pairs[pr + 1] = load_xpair(pr + 1)
            x_sb = x_pairs[pr] if half == 0 else x_pairs.pop(pr)
            base = half * KT * P
            pss = [
                ps_mm.tile([P, CW], F32, tag="ps", name=f"ps_{m}_{c}")
                for c in range(NCH)
            ]
            for kt in range(KT):
                lhsT = x_sb[:, base + kt * P : base + (kt + 1) * P]
                for c in range(NCH):
                    nc.tensor.matmul(
                        pss[c][:, :],
                        lhsT,
                        wslice(kt, c),
                        start=(kt == 0),
                        stop=(kt == KT - 1),
                    )
            out_sb = out_pool.tile([P, GO], F32, tag="outp")
            for c in range(NCH):
                nc.vector.tensor_add(
                    out=out_sb[:, c * CW : (c + 1) * CW],
                    in0=pss[c][:, :],
                    in1=bias_sb[:, c * CW : (c + 1) * CW],
                )
            # store partition halves concurrently on two queues (halves the
            # descriptor chain each queue processes -> ~3us per store)
            HP = P // 2
            nc.gpsimd.dma_start(
                out=out[m * P : m * P + HP, :], in_=out_sb[0:HP, :]
            )
            nc.sync.dma_start(
                out=out[m * P + HP : (m + 1) * P, :], in_=out_sb[HP:P, :]
            )

    nc.finalize()
    dedup_ldweights(nc)
    return nc


_NC_CACHE = {}


def _get_nc():
    if "nc" not in _NC_CACHE:
        _NC_CACHE["nc"] = build_nc()
    return _NC_CACHE["nc"]


def _prep_x(x):
    # x (4096, 1024) f32 -> xt[m, p, kt*128 + c] = x[m*128+c, kt*128+p], then
    # pack m-pairs so each DMA moves 4KB-contiguous per-partition rows
    xt = x.reshape(MT, P, KT, P).transpose(0, 3, 2, 1).reshape(MT, P, KT * P)
    xp = xt.reshape(MT // 2, 2, P, KT * P).transpose(0, 2, 1, 3)
    return np.ascontiguousarray(
        xp.reshape(MT // 2, P, 2 * KT * P).astype(ml_dtypes.bfloat16)
    )


def _prep_w(Wc):
    # Wc (GPC, 1024, 1024) [g, o, i] -> wt[p, kt, g*1024+o], i = kt*128+p, bf16
    wt = Wc.transpose(2, 0, 1).reshape(KT, P, GO)
    return np.ascontiguousarray(wt.transpose(1, 0, 2).astype(ml_dtypes.bfloat16))


def _run(inputs, trace=False):
    x = np.asarray(inputs["x"], dtype=np.float32)
    W = np.asarray(inputs["W"], dtype=np.float32)
    b = np.asarray(inputs["b"], dtype=np.float32)
    nc = _get_nc()
    xt = _prep_x(x)
    in_maps = []
    for c in range(NCORES):
        in_maps.append(
            {
                "x": xt,
                "W": _prep_w(W[c * GPC : (c + 1) * GPC]),
                "b": np.ascontiguousarray(
                    b[c * GPC : (c + 1) * GPC].reshape(1, GO)
                ),
            }
        )
    res = run_bass_kernel_spmd(nc, in_maps, core_ids=list(range(NCORES)), trace=trace)
    shards = [r["out"].reshape(BATCH, GPC, D_OUT) for r in res.results]
    return np.concatenate(shards, axis=1), res


def kernel(**inputs):
    out, _ = _run(inputs, trace=False)
    return out


# revision 11
# speedup vs baseline: 1.0405x; 1.0405x over previous
"""GroupLinear Trainium2 kernel.

out[b, g, o] = sum_i x[b, i] * W[g, o, i] + b[g, o]
  x: (4096, 1024) f32, W: (16, 1024, 1024) f32, b: (16, 1024) f32
  out: (4096, 16, 1024) f32

Sharding: groups across the 8 cores (2 groups/core), x replicated.

Per-core schedule (all operands pre-transposed host-side, cast to bf16):
  - x is staged as xt[m][i_part, kt*128 + b] (contraction dim on partitions),
    W as wt[i_part, kt, g*1024+o]. No on-device transposes at all.
  - For each batch tile m: 8 kt-steps x 4 psum chunks of 512 accumulate
    out[b, go] with x-tile stationary / W moving; bias fused into the DVE
    PSUM->SBUF evacuation.
  - A post-finalize pass deletes InstLdweights that reload the stationary
    operand already resident in the PE array (walrus emits one per matmul;
    consecutive matmuls here share lhsT so 3 of 4 reloads are redundant --
    each costs ~50-150ns of serialized PE time).
"""

import sys
import types

sys.path.insert(0, "/opt/trn_rl_repo")

# Provide antenv.axon_hooks (NTFF profile hook registry) if the installed
# antenv lacks it — must exist before the first jax/axon backend init.
try:
    from antenv import axon_hooks as _axon_hooks  # noqa: F401
except ImportError:
    _m = types.ModuleType("antenv.axon_hooks")
    _m._hook = None

    def _set_hook(hook, _m=_m):
        _m._hook = hook

    def _get_hook(_m=_m):
        return _m._hook

    _m.set_axon_ntff_profile_hook = _set_hook
    _m.get_axon_ntff_profile_hook = _get_hook
    sys.modules["antenv.axon_hooks"] = _m
    try:
        import antenv

        antenv.axon_hooks = _m
    except ImportError:
        pass

from contextlib import ExitStack

import ml_dtypes
import numpy as np

import concourse.mybir as mybir
import concourse.tile as tile
from concourse import bacc
from concourse.bass_utils import run_bass_kernel_spmd

F32 = mybir.dt.float32
BF16 = mybir.dt.bfloat16

BATCH, D_IN, D_OUT, GROUPS, NCORES = 4096, 1024, 1024, 16, 8
GPC = GROUPS // NCORES  # groups per core
P = 128
KT = D_IN // P          # contraction tiles
MT = BATCH // P         # batch tiles
GO = GPC * D_OUT        # output columns per core
CW = 512                # psum chunk width (1 bank fp32)
NCH = GO // CW          # psum chunks per batch tile


def dedup_ldweights(nc):
    """Delete InstLdweights that reload the exact stationary operand already
    resident in the PE array (same AP as the previous PE Ldweights, nothing
    in between that could clobber the array), when they carry no syncs."""
    n_removed = 0
    for blk in nc.m.functions[0].blocks:
        last_key = None
        to_remove = []
        for inst in blk.instructions:
            if getattr(inst, "engine", None) != mybir.EngineType.PE:
                continue
            tn = type(inst).__name__
            if tn == "InstLdweights":
                ap = inst.ins[0]
                key = (ap.memref, ap.offset, str(ap.ap), str(ap.dtype))
                si = inst.sync_info
                has_sync = si is not None and (
                    len(si.on_wait) > 0 or len(si.on_update) > 0
                )
                if key == last_key and not has_sync:
                    to_remove.append(inst)
                    n_removed += 1
                last_key = key
            elif tn == "InstMatmult" and inst.ldweights is False:
                continue  # non-self-loading matmul: array weights unchanged
            elif tn == "InstEventSemaphore":
                continue  # pure sync, does not touch the array
            else:
                last_key = None  # anything else: conservatively assume clobber
        for inst in to_remove:
            blk.instructions.remove(inst)
    return n_removed


def build_nc():
    nc = bacc.Bacc("TRN2", target_bir_lowering=False, debug=False)
    x = nc.dram_tensor("x", [MT, P, KT * P], BF16, kind="ExternalInput").ap()
    W = nc.dram_tensor("W", [P, KT, GO], BF16, kind="ExternalInput").ap()
    b = nc.dram_tensor("b", [1, GO], F32, kind="ExternalInput").ap()
    out = nc.dram_tensor("out", [BATCH, GO], F32, kind="ExternalOutput").ap()

    with ExitStack() as ctx:
        tc = ctx.enter_context(tile.TileContext(nc))
        singles = ctx.enter_context(tc.tile_pool(name="singles", bufs=1))
        wt_pool = ctx.enter_context(tc.tile_pool(name="wt", bufs=1))
        xin_pool = ctx.enter_context(tc.tile_pool(name="xin", bufs=3))  # m-pair tiles
        out_pool = ctx.enter_context(tc.tile_pool(name="outp", bufs=3))
        ps_mm = ctx.enter_context(tc.tile_pool(name="ps_mm", bufs=8, space="PSUM"))

        import concourse.bass as bass

        # DMA on TRN2 is descriptor-paced (~46ns per partition-row per
        # queue), so transfers are organized for fat contiguous rows:
        # W as two 4-kt quads (16KB rows), x as m-pair tiles (4KB rows),
        # out stores split by partition halves across two queues.

        # bias broadcast to all 128 partitions: [128, GO] (gpsimd queue)
        bias_sb = singles.tile([P, GO], F32)
        b_bcast = bass.AP(tensor=b.tensor, offset=b.offset, ap=[[0, P], [1, GO]])
        nc.gpsimd.dma_start(out=bias_sb[:, :], in_=b_bcast)

        # W resident in SBUF, kt-chunked loads on the sync queue; subtile
        # dependency tracking lets matmuls chase the chunks as they land
        wt = wt_pool.tile([P, KT, GO], BF16)
        for kt in range(KT):
            nc.sync.dma_start(out=wt[:, kt, :], in_=W[:, kt, :])

        def load_x(m):
            x_sb = xin_pool.tile([P, KT * P], BF16, tag="xin")
            nc.scalar.dma_start(out=x_sb[:, :], in_=x[m, :, :])
            return x_sb

        x_tiles = {0: load_x(0)}
        if MT > 1:
            x_tiles[1] = load_x(1)

        # HAM warmup: dummy matmuls on a zeroed tile while W/x stream in, so
        # the PE clock-gate is already 8/8 when real matmuls start (~13us);
        # sized to end right around then so the PE never idles a MID window.
        warm = singles.tile([P, CW], BF16)
        nc.vector.memset(warm[:, :], 0.0)
        warm_ps = ps_mm.tile([P, CW], F32, tag="ps", name="ps_warm")
        for _ in range(14):
            nc.tensor.matmul(
                warm_ps[:, :], warm[:, 0:P], warm[:, :], start=True, stop=True
            )

        for m in range(MT):
            if m + 2 < MT:
                x_tiles[m + 2] = load_x(m + 2)
            x_sb = x_tiles.pop(m)
            pss = [
                ps_mm.tile([P, CW], F32, tag="ps", name=f"ps_{m}_{c}")
                for c in range(NCH)
            ]
            for kt in range(KT):
                lhsT = x_sb[:, kt * P : (kt + 1) * P]
                for c in range(NCH):
                    nc.tensor.matmul(
                        pss[c][:, :],
                        lhsT,
                        wt[:, kt, c * CW : (c + 1) * CW],
                        start=(kt == 0),
                        stop=(kt == KT - 1),
                    )
            out_sb = out_pool.tile([P, GO], F32, tag="outp")
            for c in range(NCH):
                nc.vector.tensor_add(
                    out=out_sb[:, c * CW : (c + 1) * CW],
                    in0=pss[c][:, :],
                    in1=bias_sb[:, c * CW : (c + 1) * CW],
                )
            # store partition halves concurrently on two queues (halves the
            # descriptor chain each queue processes -> ~3us per store)
            HP = P // 2
            nc.gpsimd.dma_start(
                out=out[m * P : m * P + HP, :], in_=out_sb[0:HP, :]
            )
            nc.sync.dma_start(
                out=out[m * P + HP : (m + 1) * P, :], in_=out_sb[HP:P, :]
            )

    nc.finalize()
    dedup_ldweights(nc)
    return nc


_NC_CACHE = {}


def _get_nc():
    if "nc" not in _NC_CACHE:
        _NC_CACHE["nc"] = build_nc()
    return _NC_CACHE["nc"]


def _prep_x(x):
    # x (4096, 1024) f32 -> xt[m, p, kt*128 + c] = x[m*128+c, kt*128+p], bf16
    xt = x.reshape(MT, P, KT, P).transpose(0, 3, 2, 1)
    return np.ascontiguousarray(xt.reshape(MT, P, KT * P).astype(ml_dtypes.bfloat16))


def _prep_w(Wc):
    # Wc (GPC, 1024, 1024) [g, o, i] -> wt[p, kt, g*1024+o], i = kt*128+p, bf16
    wt = Wc.transpose(2, 0, 1).reshape(KT, P, GO)
    return np.ascontiguousarray(wt.transpose(1, 0, 2).astype(ml_dtypes.bfloat16))


def _run(inputs, trace=False):
    x = np.asarray(inputs["x"], dtype=np.float32)
    W = np.asarray(inputs["W"], dtype=np.float32)
    b = np.asarray(inputs["b"], dtype=np.float32)
    nc = _get_nc()
    xt = _prep_x(x)
    in_maps = []
    for c in range(NCORES):
        in_maps.append(
            {
                "x": xt,
                "W": _prep_w(W[c * GPC : (c + 1) * GPC]),
                "b": np.ascontiguousarray(
                    b[c * GPC : (c + 1) * GPC].reshape(1, GO)
                ),
            }
        )
    res = run_bass_kernel_spmd(nc, in_maps, core_ids=list(range(NCORES)), trace=trace)
    shards = [r["out"].reshape(BATCH, GPC, D_OUT) for r in res.results]
    return np.concatenate(shards, axis=1), res


def kernel(**inputs):
    out, _ = _run(inputs, trace=False)
    return out
